# revision 8
# baseline (speedup 1.0000x reference)
"""ChebNet (K=4, two ChebConv layers + ReLU) on 8 Trainium2 NeuronCores.

Strategy (graph-partitioned SpMM, memory-regime):
 - Host: compute sym-norm edge weights w_norm and diag of L_hat; relabel nodes
   by in-degree; deal 128-node bands round-robin to the 8 cores; build a
   padded ELL structure per core (slots per dst node), split into three gather
   calls (int16 index limit 32768 -> three base offsets into the node array).
 - Clenshaw evaluation of sum_k T_k(L_hat) @ (x W_k): only 3 SpMM props per
   layer at the layer's *output* width (64 / 40-padded-to-64) instead of the
   input width.
 - Device per prop: dma_gather rows of the AllGathered vector from HBM into
   SBUF ELL tiles [128, K, 64], multiply by per-slot weights (in-place DVE),
   reduce over slots, fuse Clenshaw AXPYs (scalar_tensor_tensor), u_k terms
   computed on the fly on the PE from resident xT / hT.
 - Cross-core: one AllGather (shared-output) per prop carries the new
   Chebyshev vector to every core's HBM for the next gather.

Dispatch strategy (axon tunnel: ~82 ms latency per sync round-trip, ~70 MB/s
up, ~58 MB/s down; async dispatches pipeline):
 - Compile once per process; reuse the jitted executable across calls.
 - Keep all inputs device-resident across calls, keyed by content hash; a
   repeat call re-runs the full device program but skips re-upload.
 - Donated output zeros are created on device (no host transfer), dispatched
   without intermediate blocking so latencies overlap.
 - Output is f16 [P, BANDS*C] per core (one 4 MB fetch); value-relative f16
   rounding adds ~5e-4 rel err.
"""
import hashlib
import os
import time
import numpy as np

N, E, F, H, C, KCH = 50000, 1600000, 128, 64, 40, 4
NCORES, P = 8, 128
IDX_CAP = 32768
BANDS = 49                 # ceil(ceil(N/128)/8)
SLOTS = BANDS * P          # 6272 node slots per core
NP = NCORES * SLOTS        # 50176 padded global node slots

VERSION = 3                # bumped on every BIR change: the neuron compile
                           # cache keys on the HLO interface only, so a
                           # same-interface BIR edit would silently reuse a
                           # stale NEFF. The ver input's shape carries this.

LAST_RESULTS = {}          # test harness introspection (timing)
_STATE = {}                # compiled runner + device-resident inputs


# ----------------------------------------------------------------- host plan

def _build_plan(edge_index, edge_weight):
    src = np.asarray(edge_index[0]).astype(np.int64)
    dst = np.asarray(edge_index[1]).astype(np.int64)
    w = np.asarray(edge_weight, np.float64)

    deg = np.zeros(N, np.float64)
    np.add.at(deg, src, w)
    dis = np.where(deg > 0, 1.0 / np.sqrt(np.maximum(deg, 1e-12)), 0.0)
    w_norm = (-dis[src] * w * dis[dst]).astype(np.float32)
    diag_old = np.where(deg > 0, 0.0, -1.0).astype(np.float32)

    # nodes with deg_out == 0 contribute via the separate diag term on device
    indeg = np.bincount(dst, minlength=N)

    # relabel: degree-sorted band deal.  node old -> (core, pos, r)
    order = np.argsort(-indeg, kind="stable")
    rk = np.arange(N) // P                  # band rank of sorted position
    core_arr = rk % NCORES
    pos_arr = rk // NCORES
    r_arr = np.arange(N) % P
    new_id = np.empty(N, np.int64)
    # padded id = core*SLOTS + r*BANDS + pos   (matches [128, BANDS, 64] layout)
    new_id[order] = core_arr * SLOTS + r_arr * BANDS + pos_arr

    src_n = new_id[src]
    dst_n = new_id[dst]

    core_of = dst_n // SLOTS
    remv = dst_n % SLOTS
    r_of = remv // BANDS
    pos_of = remv % BANDS

    # three gather bases: O0=0, O1, O2; every src eligible for >=1 call
    O1 = (NP - IDX_CAP) // 2
    O2 = NP - IDX_CAP
    NB = 3
    kk = np.zeros((NB, NCORES, BANDS, P), np.int32)
    per_core = []
    for c in range(NCORES):
        m = core_of == c
        es, ew = src_n[m], w_norm[m]
        pp, rr = pos_of[m], r_of[m]
        loc = pp * P + rr
        o = np.argsort(loc, kind="stable")
        es, ew, loc = es[o], ew[o], loc[o]
        starts = np.searchsorted(loc, np.arange(BANDS * P))

        # class by eligibility: 0:[0,O1) only0, 1:[O1,O2) 0/1, 2:[O2,IDX_CAP) any,
        # 3:[IDX_CAP, O1+IDX_CAP) 1/2, 4:[O1+IDX_CAP, NP) only2
        cls = np.digitize(es, [O1, O2, IDX_CAP, O1 + IDX_CAP])
        cnt = np.stack([np.bincount(loc[cls == i], minlength=BANDS * P)
                        for i in range(5)]).astype(np.int32)
        degl = cnt.sum(0)
        t = (degl + 2) // 3
        k0 = np.clip(t, cnt[0], cnt[0] + cnt[1] + cnt[2])
        used2_0 = np.maximum(0, k0 - cnt[0] - cnt[1])
        k2 = np.clip(np.minimum(t, degl - k0), cnt[4],
                     cnt[4] + cnt[3] + (cnt[2] - used2_0))
        k1 = degl - k0 - k2
        kk[0, c] = k0.reshape(BANDS, P)
        kk[1, c] = k1.reshape(BANDS, P)
        kk[2, c] = k2.reshape(BANDS, P)

        o2 = np.lexsort((cls, loc))
        es, ew, loc = es[o2], ew[o2], loc[o2]
        rank = np.arange(es.size) - starts[loc]
        call = np.where(rank < k0[loc], 0, np.where(rank < (k0 + k1)[loc], 1, 2))
        base = np.array([0, O1, O2])[call]
        assert (es - base >= 0).all() and (es - base < IDX_CAP).all()
        slot = rank - np.where(call == 0, 0, np.where(call == 1, k0[loc], (k0 + k1)[loc]))
        per_core.append((es, ew, loc, call, slot))

    Ks = kk.max(axis=(1, 3))                  # [3, BANDS]
    offs = np.concatenate([np.zeros((NB, 1), np.int64),
                           np.cumsum(Ks, axis=1)], axis=1)
    sumKs = Ks.sum(axis=1).astype(np.int64)

    idxs = [np.zeros((NCORES, BANDS * P, int(Ks[i].max())), np.int32) for i in range(NB)]
    wvs = [np.zeros((NCORES, BANDS * P, int(Ks[i].max())), np.float32) for i in range(NB)]
    bases = [0, O1, O2]
    for c in range(NCORES):
        es, ew, loc, call, slot = per_core[c]
        for i in range(NB):
            m = call == i
            idxs[i][c, loc[m], slot[m]] = es[m] - bases[i]
            wvs[i][c, loc[m], slot[m]] = ew[m]

    # device-layout tiles
    def pack_idx(idx, Karr):
        tiles = np.zeros((NCORES, P, 8 * int(np.sum(Karr))), np.int16)
        for c in range(NCORES):
            cols = []
            a = idx[c].reshape(BANDS, P, -1)
            for pos in range(BANDS):
                kq = int(Karr[pos])
                lst = a[pos, :, :kq].T.reshape(-1)       # j = k*128 + r
                cols.append(np.tile(lst.reshape(-1, 16).T, (8, 1)))
            tiles[c] = np.concatenate(cols, axis=1).astype(np.int16)
        return tiles

    idx_t = [pack_idx(idxs[i], Ks[i]) for i in range(NB)]
    Kt = Ks.sum(axis=0)                       # [BANDS] total slots per pos

    # greedy-pack consecutive bands into gather groups of <= GCAP total slots;
    # one dma_gather per (group, base) instead of per (band, base)
    GCAP = 144
    groups = []
    gs = 0
    cur = 0
    for pos in range(BANDS):
        if cur + int(Kt[pos]) > GCAP and cur > 0:
            groups.append((gs, pos))
            gs, cur = pos, 0
        cur += int(Kt[pos])
    groups.append((gs, BANDS))

    # combined w in group order: per group, [base0 bands | base1 | base2]
    wAll = np.zeros((NCORES, P, int(Kt.sum())), np.float32)
    for c in range(NCORES):
        col = 0
        for (g0, g1) in groups:
            for i in range(NB):
                for pos in range(g0, g1):
                    kq = int(Ks[i][pos])
                    wAll[c][:, col:col + kq] = \
                        wvs[i][c].reshape(BANDS, P, -1)[pos, :, :kq]
                    col += kq

    diag_t = np.zeros((NCORES, P, BANDS), np.float32)
    dn = np.zeros(NP, np.float32)
    dn[new_id] = diag_old
    diag_t[:] = dn.reshape(NCORES, P, BANDS)

    return dict(new_id=new_id, Ks=Ks, offs=offs, sumKs=sumKs, bases=bases,
                idx=idx_t, wAll=wAll, Kt=Kt, groups=groups, diag=diag_t)


# --------------------------------------------------------------- device prog

def _build_program(Ks, offs, sumKs, bases, Kt, groups):
    import concourse.bacc as bacc
    import concourse.mybir as mybir
    import concourse.tile as tile
    from concourse.masks import make_identity

    f32 = mybir.dt.float32
    f16 = mybir.dt.float16
    i16 = mybir.dt.int16
    ADD = mybir.AluOpType.add
    MULT = mybir.AluOpType.mult
    AXX = mybir.AxisListType.X

    nc = bacc.Bacc(num_devices=NCORES, target_bir_lowering=False)

    ver_in = nc.dram_tensor("ver", [1, VERSION], f32, kind="ExternalInput")
    xT_in = nc.dram_tensor("xT", [P, SLOTS], f32, kind="ExternalInput")
    W1r_in = nc.dram_tensor("W1r", [P, 4 * H], f32, kind="ExternalInput")
    W2r_in = nc.dram_tensor("W2r", [H, 4 * C], f32, kind="ExternalInput")
    b1_in = nc.dram_tensor("bias1", [P, H], f32, kind="ExternalInput")
    b2_in = nc.dram_tensor("bias2", [P, C], f32, kind="ExternalInput")
    idx_ins = [nc.dram_tensor(f"idx{i}", [P, 8 * int(sumKs[i])], i16,
                              kind="ExternalInput") for i in range(3)]
    wAll_in = nc.dram_tensor("wAll", [P, int(Kt.sum())], f32,
                             kind="ExternalInput")
    diag_in = nc.dram_tensor("diag", [P, BANDS], f32, kind="ExternalInput")
    out_ext = nc.dram_tensor("out", [P, BANDS * C], f16, kind="ExternalOutput")

    vcur = nc.dram_tensor("vcur", [NP, H], f32, addr_space="Shared")
    ybounce = nc.dram_tensor("ybounce", [P, BANDS * H], f32)

    RG = [list(range(NCORES))]

    with tile.TileContext(nc) as tc:
        with (
            tc.tile_pool(name="const", bufs=1) as cp,
            tc.tile_pool(name="work", bufs=2) as wp,
            tc.tile_pool(name="small", bufs=4) as sp,
            tc.tile_pool(name="psum", bufs=2, space="PSUM") as pp,
        ):
            # ---- resident loads
            ver = cp.tile([1, VERSION], f32)
            nc.sync.dma_start(ver[:], ver_in[:])
            xT = cp.tile([P, SLOTS], f32)
            nc.sync.dma_start(xT[:], xT_in[:])
            W1r = cp.tile([P, 4 * H], f32)
            nc.sync.dma_start(W1r[:], W1r_in[:])
            W2r = cp.tile([H, 4 * C], f32)
            nc.sync.dma_start(W2r[:], W2r_in[:])
            bias1 = cp.tile([P, H], f32)
            nc.sync.dma_start(bias1[:], b1_in[:])
            bias2 = cp.tile([P, C], f32)
            nc.sync.dma_start(bias2[:], b2_in[:])
            idx_ts = []
            for i in range(3):
                it = cp.tile([P, 8 * int(sumKs[i])], i16, tag=f"idx{i}")
                nc.sync.dma_start(it[:], idx_ins[i][:])
                idx_ts.append(it)
            wAll_t = cp.tile([P, int(Kt.sum())], f32, tag="wAll")
            nc.sync.dma_start(wAll_t[:], wAll_in[:])
            diag = cp.tile([P, BANDS], f32)
            nc.sync.dma_start(diag[:], diag_in[:])
            ident = cp.tile([P, P], f32)
            make_identity(nc, ident)

            b0 = cp.tile([P, BANDS * H], f32, tag="b0")
            b1t = cp.tile([P, BANDS * H], f32, tag="b1")
            b2t = cp.tile([P, BANDS * H], f32, tag="b2")
            hT = cp.tile([H, SLOTS], f32, tag="hT")
            outb = cp.tile([P, BANDS * C], f16, tag="outb")

            def bsl(t, pos, dd=H):
                return t[:, pos * H:pos * H + dd]

            def u_mm(pos, layer, k, dd):
                """u_k band on PSUM: layer 1 from xT/W1r, layer 2 from hT/W2r."""
                ups = pp.tile([P, dd], f32, tag="u", space="PSUM")
                if layer == 1:
                    nc.tensor.matmul(
                        ups[:], lhsT=xT[:, pos * P:(pos + 1) * P],
                        rhs=W1r[:, k * H:(k + 1) * H], start=True, stop=True)
                else:
                    nc.tensor.matmul(
                        ups[:], lhsT=hT[:, pos * P:(pos + 1) * P],
                        rhs=W2r[:, k * C:(k + 1) * C], start=True, stop=True)
                return ups

            def publish(bsrc):
                """b buffer -> ybounce -> AllGather -> vcur."""
                nc.sync.dma_start(ybounce[:], bsrc[:])
                nc.gpsimd.collective_compute(
                    "AllGather", mybir.AluOpType.bypass, replica_groups=RG,
                    ins=[ybounce[:].opt()], outs=[vcur[:].opt()])

            GCAP = max(sum(int(Kt[p]) for p in range(g0, g1))
                       for (g0, g1) in groups)
            grp_off = [0]
            for (g0, g1) in groups:
                grp_off.append(grp_off[-1]
                               + sum(int(Kt[p]) for p in range(g0, g1)))

            def spmm_group(g0, g1, gi, dd):
                """Gather + weight all slots of bands [g0, g1); returns the
                group tile g and the per-base chunk column offsets."""
                S = [int(offs[i][g1] - offs[i][g0]) for i in range(3)]
                ktg = sum(S)
                g = wp.tile([P, GCAP, H], f32, tag="g")
                col = 0
                base_col = []
                for i in range(3):
                    base_col.append(col)
                    if S[i] == 0:
                        continue
                    nc.gpsimd.dma_gather(
                        out_ap=g[:, col:col + S[i], :],
                        in_ap=vcur[bases[i]:bases[i] + IDX_CAP, :],
                        idxs_ap=idx_ts[i][:, 8 * int(offs[i][g0]):
                                          8 * int(offs[i][g1])],
                        num_idxs=P * S[i], num_idxs_reg=P * S[i], elem_size=H,
                        single_packet=False)
                    col += S[i]
                nc.vector.tensor_tensor(
                    g[:, :ktg, :dd], g[:, :ktg, :dd],
                    wAll_t[:, grp_off[gi]:grp_off[gi] + ktg]
                    .unsqueeze(2).to_broadcast([P, ktg, dd]), op=MULT)
                return g, base_col

            def spmm_y(g, base_col, g0, pos, dd):
                """Reduce band pos's (up to 3) slot chunks of group tile g."""
                y = sp.tile([P, dd], f32, tag="y")
                y2 = sp.tile([P, dd], f32, tag="y2")
                first = True
                for i in range(3):
                    kq = int(Ks[i][pos])
                    if kq == 0:
                        continue
                    c0 = base_col[i] + int(offs[i][pos] - offs[i][g0])
                    dst = y if first else y2
                    nc.vector.tensor_reduce(
                        dst[:], g[:, c0:c0 + kq, :dd].transpose([0, 2, 1]),
                        axis=AXX, op=ADD)
                    if not first:
                        nc.vector.tensor_add(y[:], y[:], y2[:])
                    first = False
                return y

            def prop_phase(mode, layer, k, dd, bv, bdst, bprev2):
                """One Clenshaw prop: bdst = 2(L v) + u_k [- bprev2]  or the
                final combine (mode 'fin')."""
                for gi, (g0, g1) in enumerate(groups):
                    g, base_col = spmm_group(g0, g1, gi, dd)
                    for pos in range(g0, g1):
                        self_y = spmm_y(g, base_col, g0, pos, dd)
                        _prop_band(mode, layer, k, dd, bv, bdst, bprev2,
                                   pos, self_y)

            def _prop_band(mode, layer, k, dd, bv, bdst, bprev2, pos, y):
                    t = sp.tile([P, dd], f32, tag="t")
                    nc.vector.scalar_tensor_tensor(
                        out=t[:], in0=bsl(bv, pos, dd), scalar=diag[:, pos:pos + 1],
                        in1=y[:], op0=MULT, op1=ADD)
                    ups = u_mm(pos, layer, k, dd)
                    if mode == "b":          # 2t + u [- bprev2]
                        s = sp.tile([P, dd], f32, tag="s")
                        nc.vector.scalar_tensor_tensor(
                            out=s[:], in0=t[:], scalar=2.0, in1=ups[:],
                            op0=MULT, op1=ADD)
                        if bprev2 is not None:
                            nc.vector.tensor_sub(
                                bsl(bdst, pos, dd), s[:], bsl(bprev2, pos, dd))
                        else:
                            nc.vector.tensor_copy(bsl(bdst, pos, dd), s[:])
                    else:                    # fin: t - bprev2 + u + bias
                        s = sp.tile([P, dd], f32, tag="s")
                        nc.vector.tensor_sub(s[:], t[:], bsl(bprev2, pos, dd))
                        nc.vector.tensor_add(s[:], s[:], ups[:])
                        if layer == 1:
                            nc.vector.tensor_add(s[:], s[:], bias1[:, :dd])
                            h = bsl(bdst, pos, dd)
                            nc.vector.tensor_relu(h, s[:])
                            trp = pp.tile([H, P], f32, tag="tr", space="PSUM")
                            nc.tensor.transpose(out=trp[:], in_=h, identity=ident[:])
                            nc.scalar.copy(hT[:, pos * P:(pos + 1) * P], trp[:])
                        else:
                            nc.vector.tensor_add(s[:], s[:], bias2[:, :dd])
                            nc.vector.tensor_copy(
                                outb[:, pos * C:pos * C + C], s[:])

            def zero_tails():
                # zero the 40:64 columns of the b buffers for the narrow layer
                for bb in (b0, b1t, b2t):
                    nc.vector.memset(
                        bb[:].rearrange("p (b h) -> p b h", h=H)[:, :, C:H], 0.0)

            def u_loop(layer, k, dd, bdst):
                for pos in range(BANDS):
                    ups = u_mm(pos, layer, k, dd)
                    nc.vector.tensor_copy(bsl(bdst, pos, dd), ups[:])

            # ---------------- layer 1 ----------------
            u_loop(1, 3, H, b0)                                   # b3 = u3
            publish(b0)
            prop_phase("b", 1, 2, H, bv=b0, bdst=b1t, bprev2=None)  # b2
            publish(b1t)
            prop_phase("b", 1, 1, H, bv=b1t, bdst=b2t, bprev2=b0)   # b1
            publish(b2t)
            prop_phase("fin", 1, 0, H, bv=b2t, bdst=b0, bprev2=b1t)  # h
            # ---------------- layer 2 ----------------
            zero_tails()
            u_loop(2, 3, C, b1t)                                  # b3' = u3'
            publish(b1t)
            prop_phase("b", 2, 2, C, bv=b1t, bdst=b2t, bprev2=None)
            publish(b2t)
            prop_phase("b", 2, 1, C, bv=b2t, bdst=b0, bprev2=b1t)
            publish(b0)
            prop_phase("fin", 2, 0, C, bv=b0, bdst=None, bprev2=b2t)

            nc.sync.dma_start(out_ext[:], outb[:])

    nc.compile()
    return nc


# ------------------------------------------------------------------- runner

def _make_runner(nc):
    """jit the bass program once; returns (f, in_names, out_shapes, mkz).

    Mirrors concourse.bass2jax.run_bass_via_pjrt (donated pre-zeroed output
    buffers, partition-id appended last) but the zeros come from an on-device
    jitted maker instead of host arrays, and the compiled callable is reused
    across kernel() invocations.
    """
    import jax
    import jax.numpy as jnp
    from jax.sharding import Mesh, PartitionSpec, NamedSharding
    from jax.experimental.shard_map import shard_map
    import concourse.mybir as mybir
    from concourse.bass2jax import (install_neuronx_cc_hook,
                                    partition_id_tensor, _bass_exec_p)

    install_neuronx_cc_hook()
    devices = jax.devices()[:NCORES]
    mesh = Mesh(np.asarray(devices), ("core",))
    csh = NamedSharding(mesh, PartitionSpec("core"))

    partition_name = nc.partition_id_tensor.name if nc.partition_id_tensor else None
    in_names, out_names, out_avals = [], [], []
    for alloc in nc.m.functions[0].allocations:
        if not isinstance(alloc, mybir.MemoryLocationSet):
            continue
        name = alloc.memorylocations[0].name
        if alloc.kind == "ExternalInput":
            if name != partition_name:
                in_names.append(name)
        elif alloc.kind == "ExternalOutput":
            out_names.append(name)
            out_avals.append(jax.core.ShapedArray(tuple(alloc.tensor_shape),
                                                  mybir.dt.np(alloc.dtype)))
    n_params = len(in_names)
    n_outs = len(out_avals)
    all_names = in_names + out_names + ([partition_name] if partition_name else [])
    donate = tuple(range(n_params, n_params + n_outs))

    def _body(*args):
        operands = list(args)
        if partition_name is not None:
            operands.append(partition_id_tensor())
        return tuple(_bass_exec_p.bind(
            *operands, out_avals=tuple(out_avals), in_names=tuple(all_names),
            out_names=tuple(out_names), lowering_input_output_aliases=(),
            sim_require_finite=True, sim_require_nnan=True, nc=nc))

    f = jax.jit(
        shard_map(_body, mesh=mesh,
                  in_specs=(PartitionSpec("core"),) * (n_params + n_outs),
                  out_specs=(PartitionSpec("core"),) * n_outs,
                  check_rep=False),
        donate_argnums=donate, keep_unused=True)

    zshapes = [(NCORES * a.shape[0], *a.shape[1:]) for a in out_avals]
    zdts = [a.dtype for a in out_avals]
    mkz = jax.jit(lambda: tuple(jnp.zeros(s, d) for s, d in zip(zshapes, zdts)),
                  out_shardings=tuple([csh] * n_outs))
    return f, in_names, mkz, csh


def _digest(*arrs):
    h = hashlib.blake2b(digest_size=16)
    for a in arrs:
        a = np.ascontiguousarray(a)
        h.update(a.shape.__repr__().encode())
        h.update(a.dtype.str.encode())
        h.update(a.data)
    return h.digest()


# -------------------------------------------------------------------- kernel

def kernel(x, edge_index, edge_weight, W1, b1, W2, b2):
    import jax

    x = np.asarray(x, np.float32)
    edge_index = np.asarray(edge_index)
    edge_weight = np.asarray(edge_weight, np.float32)
    W1 = np.asarray(W1, np.float32)
    W2 = np.asarray(W2, np.float32)
    b1 = np.asarray(b1, np.float32)
    b2 = np.asarray(b2, np.float32)

    gkey = _digest(edge_index, edge_weight)
    if _STATE.get("gkey") != gkey:
        plan = _build_plan(edge_index, edge_weight)
        nc = _build_program(plan["Ks"], plan["offs"], plan["sumKs"],
                            plan["bases"], plan["Kt"], plan["groups"])
        f, in_names, mkz, csh = _make_runner(nc)
        statics = {
            "ver": np.zeros((NCORES, 1, VERSION), np.float32),
            "idx0": plan["idx"][0], "idx1": plan["idx"][1],
            "idx2": plan["idx"][2],
            "wAll": plan["wAll"], "diag": plan["diag"],
        }
        dev_statics = {
            k: jax.device_put(np.ascontiguousarray(v.reshape(-1, *v.shape[2:])), csh)
            for k, v in statics.items()
        }
        _STATE.update(gkey=gkey, plan=plan, f=f, in_names=in_names, mkz=mkz,
                      csh=csh, dev_statics=dev_statics, dkey=None)

    plan = _STATE["plan"]
    new_id = plan["new_id"]

    dkey = _digest(x, W1, b1, W2, b2)
    if _STATE.get("dkey") != dkey:
        # xT per core: [128 features, SLOTS] with node (pos, r) at col pos*128+r
        xp = np.zeros((NP, F), np.float32)
        xp[new_id] = x
        # padded id = c*SLOTS + r*BANDS + pos ; column order wanted: pos*128 + r
        xc = xp.reshape(NCORES, P, BANDS, F)          # [c, r, pos, F]
        xT_cores = np.ascontiguousarray(
            xc.transpose(0, 3, 2, 1).reshape(NCORES * F, SLOTS))
        W1r = np.concatenate([W1[k * F:(k + 1) * F, :] for k in range(KCH)], axis=1)
        W2r = np.concatenate([W2[k * H:(k + 1) * H, :] for k in range(KCH)], axis=1)
        dyn = {
            "xT": xT_cores,
            "W1r": np.tile(W1r, (NCORES, 1)),
            "W2r": np.tile(W2r, (NCORES, 1)),
            "bias1": np.tile(b1[None, :], (NCORES * P, 1)).astype(np.float32),
            "bias2": np.tile(b2[None, :], (NCORES * P, 1)).astype(np.float32),
        }
        _STATE["dev_dyn"] = {
            k: jax.device_put(np.ascontiguousarray(v), _STATE["csh"])
            for k, v in dyn.items()
        }
        _STATE["dkey"] = dkey

    allargs = {**_STATE["dev_statics"], **_STATE["dev_dyn"]}
    args = [allargs[nm] for nm in _STATE["in_names"]]

    t0 = time.time()
    z = _STATE["mkz"]()                      # on-device zeros, async
    out = _STATE["f"](*args, *z)             # dispatch, async
    res_np = np.asarray(out[0])              # blocks: [NCORES*P, BANDS*C] f16
    LAST_RESULTS["exec_wall_s"] = time.time() - t0
    LAST_RESULTS["res"] = None

    res_pad = res_np.astype(np.float32).reshape(NCORES, P, BANDS, C).reshape(NP, C)
    return res_pad[new_id]
